# revision 21
# baseline (speedup 1.0000x reference)
"""Causal attention (B=4, S=2048, D=1024, single head) on 8 TRN2 NeuronCores.

Sharding: data-parallel over batch x causal-balanced query split.
  core c -> batch b = c//2, role r = c%2.
  Queries: the 16 tiles of 128 rows have causal visit-needs 1..16 key
  chunks (of 128). Role 0 takes tiles [0,3,4,7,8,11,12,15], role 1
  takes [1,2,5,6,9,10,13,14]: slot s pairs tiles with needs {2s+1,
  2s+2}, so one SPMD program with per-slot visit counts c_s = 2s+2
  covers both roles with only 4 fully-masked padding chunks per core
  (72 visited vs 68 exact; the old 512-key blocking visited 80).
  K/V: each core projects only its half of the sequence (role 0 rows
  0:1024, role 1 rows 1024:2048) and the halves are exchanged pair-wise
  through shared DRAM with tiny AllGather collectives as rendezvous.

Attention is computed in transposed layout: S^T[k, q] = (K^T)^T-free
matmul with lhsT = K^T chunk [e, k] and rhs = Q^T [e, q], kc-major over
the contiguous q-suffix of slots that visit chunk kc. exp(S^T) lands
directly in the [k, q] layout that the context matmul needs as its
stationary operand, so no PE transposes of P are required. Causality is
data-driven: only the last two chunks of each slot can straddle a
role's diagonal; they get a 128x128 additive bias (-1e6 where
kpos > qidx) built from one iota row and a host-provided per-(p, kc)
offset column. All earlier chunks are fully visible for both roles and
need no mask at all. The softmax denominator is accumulated on the PE
with N=1 ones-matmuls that reuse the context matmul's stationary P^T
chunk.

Compute is bf16 on the TensorEngine with f32 PSUM accumulation; softmax
skips the running max (logits are ~N(0,1) after the 1/32 scale; masked
lanes sit at -31250 and underflow to exactly 0).
"""

import sys

if "/opt/trn_rl_repo" not in sys.path:
    sys.path.insert(0, "/opt/trn_rl_repo")

import ml_dtypes
import numpy as np

import bass_rust

import concourse.bass as bass
import concourse.mybir as mybir
from concourse.tile import TileContext
from concourse.tile_rust import add_dep_helper

B, S, D = 4, 2048, 1024
P = 128
NCORES = 8
DC = D // P           # 8 contraction chunks of 128
QROWS = S // 2        # 1024 query rows per core
SH = S // 2           # this core's K/V half
KBLK = 512            # key block size (exchange granularity)
NKB = S // KBLK       # 4 key blocks
NKC = S // P          # 16 key chunks of 128
NSLOT = QROWS // P    # 8 query slots of 128 rows
SCALE = 1.0 / np.sqrt(np.float32(D))
MASK_NEG = -1.0e6
GROUPS = [[0, 1], [2, 3], [4, 5], [6, 7]]

# slot s visits chunks 0..CS[s]-1
CS = [2 * s + 2 for s in range(NSLOT)]
# role -> global 128-row tile index per slot (needs {2s+1, 2s+2})
TILES = {0: (0, 3, 4, 7, 8, 11, 12, 15), 1: (1, 2, 5, 6, 9, 10, 13, 14)}

F32 = mybir.dt.float32
BF16 = mybir.dt.bfloat16


# ---------------------------------------------------------------------------
# This container's walrus build (setupSyncWait, CoreV2/V3GenImpl.cpp) rejects
# any instruction carrying more than one sem wait. Tile's wait-assignment
# freely emits several. Hoist all but one wait of each instruction onto NOPs
# inserted immediately before it on the same engine — the engine executes its
# stream in order, so waiting on a preceding same-engine NOP is equivalent.
def _split_multi_waits(nc):
    n_split = 0
    for fn in nc.m.functions:
        for bb in fn.blocks:
            insts = list(bb.instructions)
            out = []
            changed = False
            for inst in insts:
                si = inst.sync_info
                if si is not None and len(si.on_wait) > 1:
                    waits = list(si.on_wait)
                    for w in waits[:-1]:
                        nop = mybir.InstNoOp(
                            name=f"{inst.name}-wsplit{n_split}", ins=[], outs=[]
                        )
                        n_split += 1
                        nop.engine = inst.engine
                        nop.sync_info = bass_rust.SyncInfo(
                            on_wait=[w], on_update=[]
                        )
                        out.append(nop)
                    inst.sync_info = bass_rust.SyncInfo(
                        on_wait=[waits[-1]], on_update=list(si.on_update)
                    )
                    changed = True
                if si is not None and len(si.on_update) > 2:
                    raise RuntimeError(
                        f"{inst.name}: {len(si.on_update)} sync updates; "
                        "update-splitting not implemented"
                    )
                out.append(inst)
            if changed:
                bb.instructions = out
    return nc
# ---------------------------------------------------------------------------


def _build_nc():
    nc = bass.Bass()

    # Host pre-permutes x^T slices and weights to [p, dc, ...] so each
    # tensor moves in ONE wide dma_start (per-partition-contiguous runs
    # -> full-size DMA packets). SP-engine dispatch is ~0.65us per
    # dma_start and strictly serial, so few big DMAs beat many small
    # ones; each dma_start's descriptors stripe over all 16 DMA engines
    # at full aggregate bandwidth regardless.
    xth = nc.declare_dram_parameter("xth", [P, DC, SH], BF16, isOutput=False)
    xqt = nc.declare_dram_parameter("xqt", [P, DC, QROWS], BF16, isOutput=False)
    wq = nc.declare_dram_parameter("wq", [P, DC, D], BF16, isOutput=False)
    wk = nc.declare_dram_parameter("wk", [P, DC, D], BF16, isOutput=False)
    wv = nc.declare_dram_parameter("wv", [P, DC, D], BF16, isOutput=False)
    qoff = nc.declare_dram_parameter("qoff", [P, NKC], F32, isOutput=False)
    rk = nc.declare_dram_parameter("rk", [1, 1], mybir.dt.uint32, isOutput=False)
    out = nc.declare_dram_parameter("out", [QROWS, D], F32, isOutput=True)

    with TileContext(nc) as tc:
        # The race-detector sim can't model pair-aliased Shared DRAM (it
        # demands a single writer); ordering for the shared exchange is
        # enforced with explicit deps instead.
        tc.race_detector_enabled = False

        # Long-lived tiles. K^T / V are per-key-block so attention only
        # waits on the specific block's collective, not the whole tensor.
        persist = tc.alloc_tile_pool(name="persist", bufs=1)
        qt_sb = persist.tile([P, DC, QROWS], BF16, tag="qt_sb")   # Q^T [e, q]
        kt_b = [
            persist.tile([P, DC, KBLK], BF16, tag=f"kt_b{v}", name=f"kt_b{v}")
            for v in range(NKB)
        ]
        v_b = [
            persist.tile([P, KBLK // P, D], BF16, tag=f"v_b{v}", name=f"v_b{v}")
            for v in range(NKB)
        ]
        iota_f = persist.tile([P, P], F32, tag="iota_f")
        qoff_sb = persist.tile([P, NKC], F32, tag="qoff_sb")
        ones_sb = persist.tile([P, 1], BF16, tag="ones_sb")

        nc.vector.memset(ones_sb[:], 1.0)

        # ---- Phase 1: projections + pair-wise K/V exchange ----
        with (
            tc.tile_pool(name="proj_in", bufs=1) as proj_in,
            tc.tile_pool(name="proj_w", bufs=2) as proj_w,
            tc.tile_pool(name="proj_st", bufs=2) as proj_st,
            tc.tile_pool(name="proj_ps", bufs=6, space="PSUM") as proj_ps,
            tc.tile_pool(name="cc_dram", bufs=1, space="DRAM") as cc_dram,
        ):
            # Dependency-free throwaway collective, emitted before anything
            # else: absorbs the ~50us (jittery) ncfw boot so the rendezvous
            # barriers below run at their ~5us post-boot cost. Collectives
            # also have a ~23us minimum spacing on this firmware, so the
            # earlier this runs the earlier the real barriers may run.
            # Barrier collectives gather garbage — no input producer needed.
            wm_in = cc_dram.tile([16], F32, tag="wm_in")
            wm_out = cc_dram.tile([2, 16], F32, tag="wm_out")
            nc.gpsimd.collective_compute(
                "AllGather",
                mybir.AluOpType.bypass,
                replica_groups=GROUPS,
                ins=[wm_in[:]],
                outs=[wm_out[:]],
            )

            xth_sb = proj_in.tile([P, DC, SH], BF16, tag="xth_sb")
            xqt_sb = proj_in.tile([P, DC, QROWS], BF16, tag="xqt_sb")

            # iota values < 128 are exact in f32
            nc.gpsimd.iota(
                iota_f[:], pattern=[[1, P]], base=0, channel_multiplier=0,
                allow_small_or_imprecise_dtypes=True,
            )

            # PE p-state warm-up: ~4us of throwaway matmuls on the iota
            # tile while the first weight/x DMAs stream in, so the real
            # projection matmuls start at full clock instead of ramping
            # through the low p-states.
            warm_ps = proj_ps.tile([P, P], F32, tag="warm", bufs=1)
            for _ in range(10):
                nc.tensor.matmul(
                    warm_ps[:], iota_f[:], iota_f[:], start=True, stop=True
                )

            # DMA dispatch order = deadline order. The very first matmul
            # group (KTh0, et=0, keys 0:128) needs only wk cols 0:128 plus
            # xth cols 0:128 (512KB) — dispatched first so the PE starts
            # ~4us earlier than a whole-tensor gate would allow. Everything
            # later is consumed behind multi-us of PE work.
            wk_sb = proj_w.tile([P, DC, D], BF16, tag="w", bufs=3)
            nc.sync.dma_start(wk_sb[:, :, 0:P], wk[:, :, 0:P])
            nc.sync.dma_start(xth_sb[:, :, 0:P], xth[:, :, 0:P])
            nc.sync.dma_start(xth_sb[:, :, P:KBLK], xth[:, :, P:KBLK])
            nc.sync.dma_start(wk_sb[:, :, P:KBLK], wk[:, :, P:KBLK])
            nc.sync.dma_start(qoff_sb[:], qoff[:, :])
            nc.sync.dma_start(wk_sb[:, :, KBLK:D], wk[:, :, KBLK:D])
            nc.sync.dma_start(xth_sb[:, :, KBLK:SH], xth[:, :, KBLK:SH])
            wv_sb = proj_w.tile([P, DC, D], BF16, tag="w", bufs=3)
            nc.sync.dma_start(wv_sb[:, :, 0:KBLK], wv[:, :, 0:KBLK])
            nc.sync.dma_start(wv_sb[:, :, KBLK:D], wv[:, :, KBLK:D])
            wq_sb = proj_w.tile([P, DC, D], BF16, tag="w", bufs=3)
            nc.sync.dma_start(wq_sb[:], wq[:, :, :])
            nc.sync.dma_start(xqt_sb[:], xqt[:, :, :])

            # K^T/V halves are exchanged through pair-shared DRAM (cores
            # 2k/2k+1 alias addr_space="Shared" allocations): each core
            # DMA-writes its stagings into its rank's slot (runtime branch
            # on the rank register — the only non-data-driven role split),
            # one tiny AllGather acts as the pair rendezvous, then both
            # halves are DMA-read back at full bandwidth. This replaces 4
            # slow data collectives (~20us/MB) with plain DMA.
            def v_half(h):
                vst = proj_st.tile(
                    [P, KBLK // P, D], BF16, tag=f"vst{h}", name=f"vst{h}", bufs=1
                )
                for ec in range(D // KBLK):
                    for st in range(KBLK // P):
                        ps = proj_ps.tile([P, KBLK], F32, tag="proj_ps")
                        for dc in range(DC):
                            nc.tensor.matmul(
                                ps[:],
                                xth_sb[:, dc, h * KBLK + st * P : h * KBLK + (st + 1) * P],
                                wv_sb[:, dc, ec * KBLK : (ec + 1) * KBLK],
                                start=(dc == 0),
                                stop=(dc == DC - 1),
                            )
                        nc.scalar.copy(vst[:, st, ec * KBLK : (ec + 1) * KBLK], ps[:])
                return vst

            def kt_half(h):
                ssl = slice(h * KBLK, (h + 1) * KBLK)
                ktst = proj_st.tile(
                    [P, DC, KBLK], BF16, tag=f"ktst{h}", name=f"ktst{h}", bufs=1
                )
                for et in range(DC):
                    ps = proj_ps.tile([P, KBLK], F32, tag="proj_ps")
                    if h == 0 and et == 0:
                        # First PE work of the program: split into 128-key
                        # sub-groups so the first matmul only waits on
                        # ~512KB of DMA (wk col-block 0 + xth cols 0:128)
                        # instead of 1.25MB.
                        for sg in range(KBLK // P):
                            for dc in range(DC):
                                nc.tensor.matmul(
                                    ps[:, sg * P : (sg + 1) * P],
                                    wk_sb[:, dc, 0:P],
                                    xth_sb[:, dc, sg * P : (sg + 1) * P],
                                    start=(dc == 0),
                                    stop=(dc == DC - 1),
                                )
                    else:
                        for dc in range(DC):
                            nc.tensor.matmul(
                                ps[:],
                                wk_sb[:, dc, et * P : (et + 1) * P],
                                xth_sb[:, dc, ssl],
                                start=(dc == 0),
                                stop=(dc == DC - 1),
                            )
                    nc.scalar.copy(ktst[:, et, :], ps[:])
                return ktst

            # One Shared tensor per (rank, slot) — the scheduler sim demands
            # a single writer inst per Shared DRAM tensor. Slots: 0=KTh0,
            # 1=Vh0, 2=KTh1, 3=Vh1 (flat 512K bf16 each).
            sh_d = [
                [
                    cc_dram.tile(
                        [D * KBLK], BF16, tag=f"sh_d{r}{j}",
                        name=f"sh_d{r}{j}", addr_space="Shared",
                    )
                    for j in range(4)
                ]
                for r in range(2)
            ]

            def kt_view(flat):
                return flat.rearrange("(et p s) -> p et s", p=P, s=KBLK)

            def v_view(flat):
                return flat.rearrange("(st p e) -> p st e", p=P, e=D)

            rk_reg = nc.sync.alloc_register("rk_reg")
            nc.sync.reg_load(rk_reg, rk[0:1, 0:1])

            # Exchange items: (staging, view, shared slot, dst tiles, half).
            # Grouping by consumption deadline: barrier #1 carries both K^T
            # halves AND V half 0 (all staged by ~57us; K blocks are consumed
            # from attention start, v_b0/v_b2 shortly after); barrier #2
            # carries only V half 1, whose blocks are consumed last — so
            # even a jittery late barrier stalls nothing.
            def exchange(name, items):
                writes = []
                for r in range(2):
                    ctx_mgr = (
                        tc.If(nc.sync.snap(rk_reg) == 0) if r == 0 else cmp.Else()
                    )
                    with ctx_mgr as branch:
                        if r == 0:
                            cmp = branch
                        for st, view, slot, _, _ in items:
                            writes.append(
                                nc.sync.dma_start(view(sh_d[r][slot]), st[:])
                            )
                b_in = cc_dram.tile(
                    [16], F32, tag=f"b_in_{name}", name=f"b_in_{name}"
                )
                b_out = cc_dram.tile(
                    [2, 16], F32, tag=f"b_out_{name}", name=f"b_out_{name}"
                )
                cc = nc.gpsimd.collective_compute(
                    "AllGather",
                    mybir.AluOpType.bypass,
                    replica_groups=GROUPS,
                    ins=[b_in[:]],
                    outs=[b_out[:]],
                )
                for w in writes:
                    add_dep_helper(cc.ins, w.ins, True, "barrier after writes")
                for rank in range(2):
                    for st, view, slot, dst, h in items:
                        rd = nc.sync.dma_start(
                            dst[2 * rank + h][:], view(sh_d[rank][slot])
                        )
                        add_dep_helper(rd.ins, cc.ins, True, "read after rdv")

            ktst0 = kt_half(0)
            ktst1 = kt_half(1)
            vst0 = v_half(0)
            exchange(
                "kkv",
                [
                    (ktst0, kt_view, 0, kt_b, 0),
                    (ktst1, kt_view, 2, kt_b, 1),
                    (vst0, v_view, 1, v_b, 0),
                ],
            )
            vst1 = v_half(1)
            exchange("v1", [(vst1, v_view, 3, v_b, 1)])

            # Q^T [e, q] = Wq^T @ xq^T (overlaps the second collective).
            # sc-outer: the first score group (q cols 0:512) only needs
            # sc=0, finished by mid-phase, so attention starts gap-free.
            for sc in range(QROWS // KBLK):
                for et in range(DC):
                    ps = proj_ps.tile([P, KBLK], F32, tag="proj_ps")
                    for dc in range(DC):
                        nc.tensor.matmul(
                            ps[:],
                            wq_sb[:, dc, et * P : (et + 1) * P],
                            xqt_sb[:, dc, sc * KBLK : (sc + 1) * KBLK],
                            start=(dc == 0),
                            stop=(dc == DC - 1),
                        )
                    nc.scalar.copy(qt_sb[:, et, sc * KBLK : (sc + 1) * KBLK], ps[:])

        # ---- Phase 2: block attention, transposed scores ----
        # Pool-open order fixes PSUM bank placement: ps_sc lands on banks
        # 6-7, which phase 1's proj_ps (banks 0-5) never touched, so the
        # first score matmul doesn't wait for the last Q^T copy to free
        # its aliased bank.
        with (
            tc.tile_pool(name="att", bufs=2) as att,
            tc.tile_pool(name="ps_lo", bufs=2, space="PSUM") as ps_lo,
            tc.tile_pool(name="ps_hi", bufs=2, space="PSUM") as ps_hi,
            tc.tile_pool(name="ps_dn", bufs=2, space="PSUM") as ps_dn,
            tc.tile_pool(name="ps_sc", bufs=2, space="PSUM") as ps_sc,
        ):
            # P^T store: [k-chunk partitions, kc, absolute q column]. Only
            # the q-suffix of slots visiting chunk kc is ever written/read.
            pt_sb = att.tile([P, NKC, QROWS], BF16, tag="pt_sb", bufs=1)

            def sc_group(kc, grps, diag):
                # scores^T for chunk kc over the q-suffix of slots that
                # visit it, in sub-groups of <=512 q columns (1 PSUM bank
                # each). When kc is the first suffix slot's diagonal
                # chunk, its 128 columns get the data-driven additive
                # mask (-1e6 where kpos > qidx) from qoff.
                kb, kr = kc // 4, (kc % 4) * P
                for gi, (a, b) in enumerate(grps):
                    n = b - a
                    ps = ps_sc.tile([P, 512], F32, tag="sc")
                    for ec in range(DC):
                        nc.tensor.matmul(
                            ps[:, 0:n],
                            kt_b[kb][:, ec, kr : kr + P],
                            qt_sb[:, ec, a:b],
                            start=(ec == 0),
                            stop=(ec == DC - 1),
                        )
                    if gi == 0 and diag:
                        bias = att.tile([P, P], F32, tag="bias")
                        nc.vector.tensor_scalar(
                            bias[:], iota_f[:], qoff_sb[:, kc : kc + 1], MASK_NEG,
                            mybir.AluOpType.is_lt, mybir.AluOpType.mult,
                        )
                        nc.vector.tensor_add(ps[:, 0:P], ps[:, 0:P], bias[:])
                    nc.scalar.activation(
                        pt_sb[:, kc, a:b], ps[:, 0:n],
                        mybir.ActivationFunctionType.Exp,
                        scale=float(SCALE),
                    )

            def ctx_slot(s, cs):
                qsl = slice(s * P, (s + 1) * P)
                lo = ps_lo.tile([P, KBLK], F32, tag="lo")
                hi = ps_hi.tile([P, KBLK], F32, tag="hi")
                dn = ps_dn.tile([P, 1], F32, tag="dn")
                for kc in range(cs):
                    ptc = pt_sb[:, kc, qsl]
                    vb = v_b[kc // 4]
                    vrow = kc % 4
                    st, sp = (kc == 0), (kc == cs - 1)
                    nc.tensor.matmul(
                        dn[:], ptc, ones_sb[:], start=st, stop=sp
                    )
                    nc.tensor.matmul(
                        lo[:], ptc, vb[:, vrow, 0:KBLK], start=st, stop=sp
                    )
                    nc.tensor.matmul(
                        hi[:], ptc, vb[:, vrow, KBLK:D], start=st, stop=sp
                    )
                rinv = att.tile([P, 1], F32, tag="rinv")
                nc.vector.reciprocal(rinv[:], dn[:])
                out_sb = att.tile([P, D], F32, tag="out_sb")
                nc.vector.tensor_scalar_mul(out_sb[:, 0:KBLK], lo[:], rinv[:])
                nc.sync.dma_start(out[s * P : (s + 1) * P, 0:KBLK], out_sb[:, 0:KBLK])
                nc.vector.tensor_scalar_mul(out_sb[:, KBLK:D], hi[:], rinv[:])
                nc.sync.dma_start(out[s * P : (s + 1) * P, KBLK:D], out_sb[:, KBLK:D])

            # Uniform SPMD schedule: slot s visits CS[s] = 2s+2 chunks
            # (the max of its two roles' needs; 4 fully-masked padding
            # chunks per core). A role-branched exact schedule would
            # save those 4 chunks but deadlocks: Tile's semaphore
            # thresholds don't support dependency chains inside
            # asymmetric If/Else branches.
            def grps_for(kc):
                return [
                    (a, min(a + 512, QROWS))
                    for a in range((kc // 2) * P, QROWS, 512)
                ]

            # Emission order keeps the PE two score-groups ahead of the
            # exp consumer before each ctx batch, hiding DVE/Act latency.
            for kc in range(4):
                sc_group(kc, grps_for(kc), True)
            ctx_slot(0, CS[0])
            for s in range(1, NSLOT - 1):
                sc_group(2 * s + 2, grps_for(2 * s + 2), True)
                sc_group(2 * s + 3, grps_for(2 * s + 3), True)
                ctx_slot(s, CS[s])
            ctx_slot(NSLOT - 1, CS[NSLOT - 1])

        persist.release()

    return _split_multi_waits(nc)


_NC_CACHE = None


def _get_nc():
    global _NC_CACHE
    if _NC_CACHE is None:
        _NC_CACHE = _build_nc()
    return _NC_CACHE


def _qrows(role):
    # 128-row tiles ordered by slot (ascending visit-need 2s+1 / 2s+2).
    return np.concatenate(
        [np.arange(t * P, (t + 1) * P) for t in TILES[role]]
    )


def _qoff(role):
    # qoff[p, kc] = kc*128 + p - qbase(slot kc//2): the per-partition
    # threshold t such that column f of the masked 128-block is causally
    # masked iff f < t (kpos > qidx).
    p = np.arange(P)[:, None]
    kc = np.arange(NKC)[None, :]
    qbase = np.array([TILES[role][k // 2] * P for k in range(NKC)])[None, :]
    return (kc * P + p - qbase).astype(np.float32)


def _perm(a):
    # [D, n] -> [p, dc, n] with [p, dc, n] = a[dc*128 + p, n]
    return np.ascontiguousarray(a.reshape(DC, P, -1).transpose(1, 0, 2))


def _shard_inputs(x, Wq, Wk, Wv):
    bf = ml_dtypes.bfloat16
    w = {
        "wq": _perm(Wq.astype(bf)),
        "wk": _perm(Wk.astype(bf)),
        "wv": _perm(Wv.astype(bf)),
    }
    qoffs = {r: _qoff(r) for r in range(2)}
    in_maps = []
    for c in range(NCORES):
        b, r = c // 2, c % 2
        rows = _qrows(r)
        xbT = x[b].T.astype(bf)                                  # [D, S]
        in_maps.append(
            {
                "xth": _perm(xbT[:, r * SH : (r + 1) * SH]),
                "xqt": _perm(xbT[:, rows]),
                "qoff": qoffs[r],
                "rk": np.array([[r]], dtype=np.uint32),
                **w,
            }
        )
    return in_maps


def _unshard(results, dtype):
    out = np.empty((B, S, D), dtype=dtype)
    for c in range(NCORES):
        b, r = c // 2, c % 2
        out[b, _qrows(r), :] = results[c]["out"]
    return out


def run(x, Wq, Wk, Wv, trace=False, tmpdir=None):
    from concourse.bass_utils import run_bass_kernel_spmd

    nc = _get_nc()
    in_maps = _shard_inputs(x, Wq, Wk, Wv)
    res = run_bass_kernel_spmd(
        nc, in_maps, core_ids=list(range(NCORES)), trace=trace, tmpdir=tmpdir
    )
    return _unshard(res.results, np.dtype(x.dtype)), res


def kernel(x, Wq, Wk, Wv):
    out, _ = run(np.asarray(x), np.asarray(Wq), np.asarray(Wk), np.asarray(Wv))
    return out


# revision 29
# speedup vs baseline: 2.0177x; 2.0177x over previous
"""Causal attention (B=4, S=2048, D=1024, single head) on 8 TRN2 NeuronCores.

Sharding: data-parallel over batch x causal-balanced query split.
  core c -> batch b = c//2, role r = c%2.
  Queries: the 16 tiles of 128 rows have causal visit-needs 1..16 key
  chunks (of 128). Role 0 takes tiles [0,3,4,7,8,11,12,15], role 1
  takes [1,2,5,6,9,10,13,14]: slot s pairs tiles with needs {2s+1,
  2s+2}, so one SPMD program with per-slot visit counts c_s = 2s+2
  covers both roles with only 4 fully-masked padding chunks per core
  (72 visited vs 68 exact; the old 512-key blocking visited 80).
  K/V: each core projects only its half of the sequence (role 0 rows
  0:1024, role 1 rows 1024:2048) and the halves are exchanged pair-wise
  through shared DRAM with tiny AllGather collectives as rendezvous.

Attention is computed in transposed layout: S^T[k, q] = (K^T)^T-free
matmul with lhsT = K^T chunk [e, k] and rhs = Q^T [e, q], kc-major over
the contiguous q-suffix of slots that visit chunk kc. exp(S^T) lands
directly in the [k, q] layout that the context matmul needs as its
stationary operand, so no PE transposes of P are required. Causality is
data-driven: only the last two chunks of each slot can straddle a
role's diagonal; they get a 128x128 additive bias (-1e6 where
kpos > qidx) built from one iota row and a host-provided per-(p, kc)
offset column. All earlier chunks are fully visible for both roles and
need no mask at all. The softmax denominator is accumulated on the PE
with N=1 ones-matmuls that reuse the context matmul's stationary P^T
chunk.

Compute is bf16 on the TensorEngine with f32 PSUM accumulation; softmax
skips the running max (logits are ~N(0,1) after the 1/32 scale; masked
lanes sit at -31250 and underflow to exactly 0).
"""

import sys

if "/opt/trn_rl_repo" not in sys.path:
    sys.path.insert(0, "/opt/trn_rl_repo")

import ml_dtypes
import numpy as np

import bass_rust

import concourse.bass as bass
import concourse.mybir as mybir
from concourse.tile import TileContext
from concourse.tile_rust import add_dep_helper

B, S, D = 4, 2048, 1024
P = 128
NCORES = 8
DC = D // P           # 8 contraction chunks of 128
QROWS = S // 2        # 1024 query rows per core
SH = S // 2           # this core's K/V half
KBLK = 512            # key block size (exchange granularity)
NKB = S // KBLK       # 4 key blocks
NKC = S // P          # 16 key chunks of 128
NSLOT = QROWS // P    # 8 query slots of 128 rows
SCALE = 1.0 / np.sqrt(np.float32(D))
MASK_NEG = -1.0e6
GROUPS = [[0, 1], [2, 3], [4, 5], [6, 7]]

# slot s visits chunks 0..CS[s]-1
CS = [2 * s + 2 for s in range(NSLOT)]
# role -> global 128-row tile index per slot (needs {2s+1, 2s+2})
TILES = {0: (0, 3, 4, 7, 8, 11, 12, 15), 1: (1, 2, 5, 6, 9, 10, 13, 14)}

F32 = mybir.dt.float32
BF16 = mybir.dt.bfloat16


# ---------------------------------------------------------------------------
# This container's walrus build (setupSyncWait, CoreV2/V3GenImpl.cpp) rejects
# any instruction carrying more than one sem wait. Tile's wait-assignment
# freely emits several. Hoist all but one wait of each instruction onto NOPs
# inserted immediately before it on the same engine — the engine executes its
# stream in order, so waiting on a preceding same-engine NOP is equivalent.
def _split_multi_waits(nc):
    n_split = 0
    for fn in nc.m.functions:
        for bb in fn.blocks:
            insts = list(bb.instructions)
            out = []
            changed = False
            for inst in insts:
                si = inst.sync_info
                if si is not None and len(si.on_wait) > 1:
                    waits = list(si.on_wait)
                    for w in waits[:-1]:
                        nop = mybir.InstNoOp(
                            name=f"{inst.name}-wsplit{n_split}", ins=[], outs=[]
                        )
                        n_split += 1
                        nop.engine = inst.engine
                        nop.sync_info = bass_rust.SyncInfo(
                            on_wait=[w], on_update=[]
                        )
                        out.append(nop)
                    inst.sync_info = bass_rust.SyncInfo(
                        on_wait=[waits[-1]], on_update=list(si.on_update)
                    )
                    changed = True
                if si is not None and len(si.on_update) > 2:
                    raise RuntimeError(
                        f"{inst.name}: {len(si.on_update)} sync updates; "
                        "update-splitting not implemented"
                    )
                out.append(inst)
            if changed:
                bb.instructions = out
    return nc
# ---------------------------------------------------------------------------


def _build_nc():
    nc = bass.Bass()

    # Host pre-permutes x^T slices and weights to [p, blk, dc, blkcols]
    # (block-major) so every dma_start moves per-partition-contiguous
    # 2KB runs -> full-size DMA packets at full aggregate bandwidth.
    # SP-engine dispatch is ~0.65us per dma_start and strictly serial,
    # so few big DMAs beat many small ones; each dma_start's descriptors
    # stripe over all 16 DMA engines regardless.
    xth = nc.declare_dram_parameter("xth", [P, SH // P, DC, P], BF16, isOutput=False)
    xqt = nc.declare_dram_parameter("xqt", [P, QROWS // P, DC, P], BF16, isOutput=False)
    wq = nc.declare_dram_parameter("wq", [P, DC, DC, P], BF16, isOutput=False)
    wk = nc.declare_dram_parameter("wk", [P, DC, DC, P], BF16, isOutput=False)
    wv = nc.declare_dram_parameter("wv", [P, 2, DC, KBLK], BF16, isOutput=False)
    qoff = nc.declare_dram_parameter("qoff", [P, NKC], F32, isOutput=False)
    rk = nc.declare_dram_parameter("rk", [1, 1], mybir.dt.uint32, isOutput=False)
    out = nc.declare_dram_parameter("out", [QROWS, D], F32, isOutput=True)

    with TileContext(nc) as tc:
        # The race-detector sim can't model pair-aliased Shared DRAM (it
        # demands a single writer); ordering for the shared exchange is
        # enforced with explicit deps instead.
        tc.race_detector_enabled = False

        # Long-lived tiles. K^T / V are per-key-block so attention only
        # waits on the specific block's collective, not the whole tensor.
        persist = tc.alloc_tile_pool(name="persist", bufs=1)
        qt_sb = persist.tile([P, DC, QROWS], BF16, tag="qt_sb")   # Q^T [e, q]
        kt_b = [
            persist.tile([P, DC, KBLK], BF16, tag=f"kt_b{v}", name=f"kt_b{v}")
            for v in range(NKB)
        ]
        v_b = [
            persist.tile([P, KBLK // P, D], BF16, tag=f"v_b{v}", name=f"v_b{v}")
            for v in range(NKB)
        ]
        iota_f = persist.tile([P, P], F32, tag="iota_f")
        qoff_sb = persist.tile([P, NKC], F32, tag="qoff_sb")
        ones_sb = persist.tile([P, 1], BF16, tag="ones_sb")

        nc.vector.memset(ones_sb[:], 1.0)

        # ---- Phase 1: projections + pair-wise K/V exchange ----
        with (
            tc.tile_pool(name="proj_in", bufs=1) as proj_in,
            tc.tile_pool(name="proj_w", bufs=2) as proj_w,
            tc.tile_pool(name="proj_st", bufs=2) as proj_st,
            tc.tile_pool(name="proj_ps", bufs=6, space="PSUM") as proj_ps,
            tc.tile_pool(name="cc_dram", bufs=1, space="DRAM") as cc_dram,
        ):
            # Dependency-free throwaway collective, emitted before anything
            # else: absorbs the ~50us (jittery) ncfw boot so the rendezvous
            # barriers below run at their ~5us post-boot cost. Collectives
            # also have a ~23us minimum spacing on this firmware, so the
            # earlier this runs the earlier the real barriers may run.
            # Barrier collectives gather garbage — no input producer needed.
            wm_in = cc_dram.tile([16], F32, tag="wm_in")
            wm_out = cc_dram.tile([2, 16], F32, tag="wm_out")
            nc.gpsimd.collective_compute(
                "AllGather",
                mybir.AluOpType.bypass,
                replica_groups=GROUPS,
                ins=[wm_in[:]],
                outs=[wm_out[:]],
            )

            xth_sb = proj_in.tile([P, SH // P, DC, P], BF16, tag="xth_sb")
            xqt_sb = proj_in.tile([P, QROWS // P, DC, P], BF16, tag="xqt_sb")

            # iota values < 128 are exact in f32
            nc.gpsimd.iota(
                iota_f[:], pattern=[[1, P]], base=0, channel_multiplier=0,
                allow_small_or_imprecise_dtypes=True,
            )


            # DMA dispatch order = deadline order. The very first matmul
            # group (KTh0, et=0, keys 0:128) needs only wk e-block 0 plus
            # xth s-block 0 (512KB) — dispatched first so the PE starts
            # ~3us earlier than a whole-tensor gate would allow. Everything
            # later is consumed behind multi-us of PE work.
            wk_sb = proj_w.tile([P, DC, DC, P], BF16, tag="w", bufs=3)
            nc.sync.dma_start(wk_sb[:, 0, :, :], wk[:, 0, :, :])
            nc.sync.dma_start(xth_sb[:, 0, :, :], xth[:, 0, :, :])
            nc.sync.dma_start(xth_sb[:, 1:4, :, :], xth[:, 1:4, :, :])
            nc.sync.dma_start(wk_sb[:, 1:4, :, :], wk[:, 1:4, :, :])
            nc.sync.dma_start(qoff_sb[:], qoff[:, :])
            nc.sync.dma_start(wk_sb[:, 4:8, :, :], wk[:, 4:8, :, :])
            nc.sync.dma_start(xth_sb[:, 4:8, :, :], xth[:, 4:8, :, :])
            wv_sb = proj_w.tile([P, 2, DC, KBLK], BF16, tag="w", bufs=3)
            nc.sync.dma_start(wv_sb[:, 0, :, :], wv[:, 0, :, :])
            nc.sync.dma_start(wv_sb[:, 1, :, :], wv[:, 1, :, :])
            wq_sb = proj_w.tile([P, DC, DC, P], BF16, tag="w", bufs=3)
            nc.sync.dma_start(wq_sb[:], wq[:, :, :, :])
            nc.sync.dma_start(xqt_sb[:], xqt[:, :, :, :])

            # K^T/V halves are exchanged through pair-shared DRAM (cores
            # 2k/2k+1 alias addr_space="Shared" allocations): each core
            # DMA-writes its stagings into its rank's slot (runtime branch
            # on the rank register — the only non-data-driven role split),
            # one tiny AllGather acts as the pair rendezvous, then both
            # halves are DMA-read back at full bandwidth. This replaces 4
            # slow data collectives (~20us/MB) with plain DMA.
            def v_half(h):
                vst = proj_st.tile(
                    [P, KBLK // P, D], BF16, tag=f"vst{h}", name=f"vst{h}", bufs=1
                )
                for ec in range(D // KBLK):
                    for st in range(KBLK // P):
                        ps = proj_ps.tile([P, KBLK], F32, tag="proj_ps")
                        for dc in range(DC):
                            nc.tensor.matmul(
                                ps[:],
                                xth_sb[:, 4 * h + st, dc, :],
                                wv_sb[:, ec, dc, :],
                                start=(dc == 0),
                                stop=(dc == DC - 1),
                            )
                        nc.scalar.copy(vst[:, st, ec * KBLK : (ec + 1) * KBLK], ps[:])
                return vst

            def kt_half(h):
                ktst = proj_st.tile(
                    [P, DC, KBLK], BF16, tag=f"ktst{h}", name=f"ktst{h}", bufs=1
                )
                for et in range(DC):
                    ps = proj_ps.tile([P, KBLK], F32, tag="proj_ps")
                    if h == 0 and et == 0:
                        # First PE work of the program: split into 128-key
                        # sub-groups so the first matmul only waits on
                        # ~512KB of DMA (wk e-block 0 + xth s-block 0)
                        # instead of 1.25MB.
                        for sg in range(KBLK // P):
                            for dc in range(DC):
                                nc.tensor.matmul(
                                    ps[:, sg * P : (sg + 1) * P],
                                    wk_sb[:, 0, dc, :],
                                    xth_sb[:, sg, dc, :],
                                    start=(dc == 0),
                                    stop=(dc == DC - 1),
                                )
                    else:
                        for dc in range(DC):
                            nc.tensor.matmul(
                                ps[:],
                                wk_sb[:, et, dc, :],
                                xth_sb[:, 4 * h : 4 * h + 4, dc, :],
                                start=(dc == 0),
                                stop=(dc == DC - 1),
                            )
                    nc.scalar.copy(ktst[:, et, :], ps[:])
                return ktst

            # One Shared tensor per (rank, slot) — the scheduler sim demands
            # a single writer inst per Shared DRAM tensor. Slots: 0=KTh0,
            # 1=Vh0, 2=KTh1, 3=Vh1 (flat 512K bf16 each).
            sh_d = [
                [
                    cc_dram.tile(
                        [D * KBLK], BF16, tag=f"sh_d{r}{j}",
                        name=f"sh_d{r}{j}", addr_space="Shared",
                    )
                    for j in range(4)
                ]
                for r in range(2)
            ]

            def kt_view(flat):
                return flat.rearrange("(et p s) -> p et s", p=P, s=KBLK)

            def v_view(flat):
                return flat.rearrange("(st p e) -> p st e", p=P, e=D)

            rk_reg = nc.sync.alloc_register("rk_reg")
            nc.sync.reg_load(rk_reg, rk[0:1, 0:1])

            # Exchange items: (staging, view, shared slot, dst tiles, half).
            # Grouping by consumption deadline: barrier #1 carries both K^T
            # halves AND V half 0 (all staged by ~57us; K blocks are consumed
            # from attention start, v_b0/v_b2 shortly after); barrier #2
            # carries only V half 1, whose blocks are consumed last — so
            # even a jittery late barrier stalls nothing.
            def exchange(name, items):
                writes = []
                for r in range(2):
                    ctx_mgr = (
                        tc.If(nc.sync.snap(rk_reg) == 0) if r == 0 else cmp.Else()
                    )
                    with ctx_mgr as branch:
                        if r == 0:
                            cmp = branch
                        for st, view, slot, _, _ in items:
                            writes.append(
                                nc.sync.dma_start(view(sh_d[r][slot]), st[:])
                            )
                b_in = cc_dram.tile(
                    [16], F32, tag=f"b_in_{name}", name=f"b_in_{name}"
                )
                b_out = cc_dram.tile(
                    [2, 16], F32, tag=f"b_out_{name}", name=f"b_out_{name}"
                )
                cc = nc.gpsimd.collective_compute(
                    "AllGather",
                    mybir.AluOpType.bypass,
                    replica_groups=GROUPS,
                    ins=[b_in[:]],
                    outs=[b_out[:]],
                )
                for w in writes:
                    add_dep_helper(cc.ins, w.ins, True, "barrier after writes")
                for rank in range(2):
                    for st, view, slot, dst, h in items:
                        rd = nc.sync.dma_start(
                            dst[2 * rank + h][:], view(sh_d[rank][slot])
                        )
                        add_dep_helper(rd.ins, cc.ins, True, "read after rdv")

            ktst0 = kt_half(0)
            ktst1 = kt_half(1)
            vst0 = v_half(0)
            exchange(
                "kkv",
                [
                    (ktst0, kt_view, 0, kt_b, 0),
                    (ktst1, kt_view, 2, kt_b, 1),
                    (vst0, v_view, 1, v_b, 0),
                ],
            )
            vst1 = v_half(1)
            exchange("v1", [(vst1, v_view, 3, v_b, 1)])

            # Q^T [e, q] = Wq^T @ xq^T (overlaps the second collective).
            # sc-outer: the first score group (q cols 0:512) only needs
            # sc=0, finished by mid-phase, so attention starts gap-free.
            for sc in range(QROWS // KBLK):
                for et in range(DC):
                    ps = proj_ps.tile([P, KBLK], F32, tag="proj_ps")
                    for dc in range(DC):
                        nc.tensor.matmul(
                            ps[:],
                            wq_sb[:, et, dc, :],
                            xqt_sb[:, 4 * sc : 4 * sc + 4, dc, :],
                            start=(dc == 0),
                            stop=(dc == DC - 1),
                        )
                    nc.scalar.copy(qt_sb[:, et, sc * KBLK : (sc + 1) * KBLK], ps[:])

        # ---- Phase 2: block attention, transposed scores ----
        # Pool-open order fixes PSUM bank placement: ps_sc lands on banks
        # 6-7, which phase 1's proj_ps (banks 0-5) never touched, so the
        # first score matmul doesn't wait for the last Q^T copy to free
        # its aliased bank.
        with (
            tc.tile_pool(name="att", bufs=2) as att,
            tc.tile_pool(name="ps_lo", bufs=2, space="PSUM") as ps_lo,
            tc.tile_pool(name="ps_hi", bufs=2, space="PSUM") as ps_hi,
            tc.tile_pool(name="ps_dn", bufs=2, space="PSUM") as ps_dn,
            tc.tile_pool(name="ps_sc", bufs=2, space="PSUM") as ps_sc,
        ):
            # P^T store: [k-chunk partitions, kc, absolute q column]. Only
            # the q-suffix of slots visiting chunk kc is ever written/read.
            pt_sb = att.tile([P, NKC, QROWS], BF16, tag="pt_sb", bufs=1)

            def sc_group(kc, grps, diag):
                # scores^T for chunk kc over the q-suffix of slots that
                # visit it, in sub-groups of <=512 q columns (1 PSUM bank
                # each). When kc is the first suffix slot's diagonal
                # chunk, its 128 columns get the data-driven additive
                # mask (-1e6 where kpos > qidx) from qoff.
                kb, kr = kc // 4, (kc % 4) * P
                for gi, (a, b) in enumerate(grps):
                    n = b - a
                    ps = ps_sc.tile([P, 512], F32, tag="sc")
                    for ec in range(DC):
                        nc.tensor.matmul(
                            ps[:, 0:n],
                            kt_b[kb][:, ec, kr : kr + P],
                            qt_sb[:, ec, a:b],
                            start=(ec == 0),
                            stop=(ec == DC - 1),
                        )
                    if gi == 0 and diag:
                        bias = att.tile([P, P], F32, tag="bias")
                        nc.vector.tensor_scalar(
                            bias[:], iota_f[:], qoff_sb[:, kc : kc + 1], MASK_NEG,
                            mybir.AluOpType.is_lt, mybir.AluOpType.mult,
                        )
                        nc.vector.tensor_add(ps[:, 0:P], ps[:, 0:P], bias[:])
                    nc.scalar.activation(
                        pt_sb[:, kc, a:b], ps[:, 0:n],
                        mybir.ActivationFunctionType.Exp,
                        scale=float(SCALE),
                    )

            def ctx_slot(s, cs):
                qsl = slice(s * P, (s + 1) * P)
                lo = ps_lo.tile([P, KBLK], F32, tag="lo")
                hi = ps_hi.tile([P, KBLK], F32, tag="hi")
                dn = ps_dn.tile([P, 1], F32, tag="dn")
                for kc in range(cs):
                    ptc = pt_sb[:, kc, qsl]
                    vb = v_b[kc // 4]
                    vrow = kc % 4
                    st, sp = (kc == 0), (kc == cs - 1)
                    nc.tensor.matmul(
                        dn[:], ptc, ones_sb[:], start=st, stop=sp
                    )
                    nc.tensor.matmul(
                        lo[:], ptc, vb[:, vrow, 0:KBLK], start=st, stop=sp
                    )
                    nc.tensor.matmul(
                        hi[:], ptc, vb[:, vrow, KBLK:D], start=st, stop=sp
                    )
                rinv = att.tile([P, 1], F32, tag="rinv")
                nc.vector.reciprocal(rinv[:], dn[:])
                out_sb = att.tile([P, D], F32, tag="out_sb")
                nc.vector.tensor_scalar_mul(out_sb[:, 0:KBLK], lo[:], rinv[:])
                nc.sync.dma_start(out[s * P : (s + 1) * P, 0:KBLK], out_sb[:, 0:KBLK])
                nc.vector.tensor_scalar_mul(out_sb[:, KBLK:D], hi[:], rinv[:])
                nc.sync.dma_start(out[s * P : (s + 1) * P, KBLK:D], out_sb[:, KBLK:D])

            # Uniform SPMD schedule: slot s visits CS[s] = 2s+2 chunks
            # (the max of its two roles' needs; 4 fully-masked padding
            # chunks per core). A role-branched exact schedule would
            # save those 4 chunks but deadlocks: Tile's semaphore
            # thresholds don't support dependency chains inside
            # asymmetric If/Else branches.
            def grps_for(kc):
                return [
                    (a, min(a + 512, QROWS))
                    for a in range((kc // 2) * P, QROWS, 512)
                ]

            # Emission order keeps the PE two score-groups ahead of the
            # exp consumer before each ctx batch, hiding DVE/Act latency.
            for kc in range(4):
                sc_group(kc, grps_for(kc), True)
            ctx_slot(0, CS[0])
            for s in range(1, NSLOT - 1):
                sc_group(2 * s + 2, grps_for(2 * s + 2), True)
                sc_group(2 * s + 3, grps_for(2 * s + 3), True)
                ctx_slot(s, CS[s])
            ctx_slot(NSLOT - 1, CS[NSLOT - 1])

        persist.release()

    return _split_multi_waits(nc)


_NC_CACHE = None


def _get_nc():
    global _NC_CACHE
    if _NC_CACHE is None:
        _NC_CACHE = _build_nc()
    return _NC_CACHE


def _qrows(role):
    # 128-row tiles ordered by slot (ascending visit-need 2s+1 / 2s+2).
    return np.concatenate(
        [np.arange(t * P, (t + 1) * P) for t in TILES[role]]
    )


def _qoff(role):
    # qoff[p, kc] = kc*128 + p - qbase(slot kc//2): the per-partition
    # threshold t such that column f of the masked 128-block is causally
    # masked iff f < t (kpos > qidx).
    p = np.arange(P)[:, None]
    kc = np.arange(NKC)[None, :]
    qbase = np.array([TILES[role][k // 2] * P for k in range(NKC)])[None, :]
    return (kc * P + p - qbase).astype(np.float32)


def _perm(a, blk):
    # [D, n] -> [p, nb, dc, blk] with value a[dc*128 + p, nb*blk + j]
    n = a.shape[1]
    return np.ascontiguousarray(
        a.reshape(DC, P, n // blk, blk).transpose(1, 2, 0, 3)
    )


def _shard_inputs(x, Wq, Wk, Wv):
    bf = ml_dtypes.bfloat16
    w = {
        "wq": _perm(Wq.astype(bf), P),
        "wk": _perm(Wk.astype(bf), P),
        "wv": _perm(Wv.astype(bf), KBLK),
    }
    qoffs = {r: _qoff(r) for r in range(2)}
    in_maps = []
    for c in range(NCORES):
        b, r = c // 2, c % 2
        rows = _qrows(r)
        xbT = x[b].T.astype(bf)                                  # [D, S]
        in_maps.append(
            {
                "xth": _perm(xbT[:, r * SH : (r + 1) * SH], P),
                "xqt": _perm(xbT[:, rows], P),
                "qoff": qoffs[r],
                "rk": np.array([[r]], dtype=np.uint32),
                **w,
            }
        )
    return in_maps


def _unshard(results, dtype):
    out = np.empty((B, S, D), dtype=dtype)
    for c in range(NCORES):
        b, r = c // 2, c % 2
        out[b, _qrows(r), :] = results[c]["out"]
    return out


def run(x, Wq, Wk, Wv, trace=False, tmpdir=None):
    from concourse.bass_utils import run_bass_kernel_spmd

    nc = _get_nc()
    in_maps = _shard_inputs(x, Wq, Wk, Wv)
    res = run_bass_kernel_spmd(
        nc, in_maps, core_ids=list(range(NCORES)), trace=trace, tmpdir=tmpdir
    )
    return _unshard(res.results, np.dtype(x.dtype)), res


def kernel(x, Wq, Wk, Wv):
    out, _ = run(np.asarray(x), np.asarray(Wq), np.asarray(Wk), np.asarray(Wv))
    return out
